# revision 39
# baseline (speedup 1.0000x reference)
"""AttentionSharingUnit on 8 Trainium2 cores (Bass/Tile).

Sharding: data-parallel over (b, d/4) -> 8 cores, zero collectives.
Each core handles one b (of 2) and one d-quarter (512 of 2048 rows) for
both frames. K/V are computed redundantly over the full 2048 keys per
core; everything else is owned-rows only.

All activations are kept feature-major ("transposed", [c, rows]) so that
every projection is a plain out[c_out, rows] = W^T-chunks (stationary)
x X^T (moving) matmul and per-feature biases are per-partition scalars.
Matmuls run in bf16 with fp32 PSUM accumulation (tolerance 2e-2).

Key identity: out = mhf + xo - h = o_spatial + xo_temporal (h cancels),
so the final output never touches h; mhf = h + o is only needed for LN.
"""

import os
import numpy as np
import ml_dtypes

DEBUG = bool(int(os.environ.get("K_DEBUG", "0")))

FRAMES = 2
HEADS = 20
C = 1280
RANK = 256
B = 2
D = 2048
EPS = 1e-6

P = 128
KC = C // P          # 10 c-chunks
RC = RANK // P       # 2 rank-chunks
DQ = 512             # d-quarter rows per core per frame
SC_N = D // P        # 16 key chunks
DH = 64              # head dim
NCORES = 8
SLOT = DH + 1        # V slot width (64 V cols + 1 ones col)

BF16 = ml_dtypes.bfloat16

_CACHE = {}


def _build():
    """Build the SPMD Bass/Tile program (same program for all 8 cores)."""
    if "nc" in _CACHE:
        return _CACHE["nc"]

    from contextlib import ExitStack
    import concourse.tile as tile
    from concourse import bacc, mybir

    F32 = mybir.dt.float32
    BF = mybir.dt.bfloat16
    AF = mybir.ActivationFunctionType

    nc = bacc.Bacc("TRN2", target_bir_lowering=False, debug=False,
                   num_devices=NCORES)

    hT = nc.dram_tensor("hT", [FRAMES, C, D], BF, kind="ExternalInput").ap()
    hTq = nc.dram_tensor("hTq", [FRAMES, C, DQ], BF, kind="ExternalInput").ap()
    wnames = ["wq", "wk", "wv", "wo", "wtq", "wtk", "wtv", "wto"]
    wT = {n: nc.dram_tensor(n + "T", [C, C], BF, kind="ExternalInput").ap()
          for n in wnames}
    dT = {n: nc.dram_tensor("d" + n + "T", [FRAMES, C, RANK], BF,
                            kind="ExternalInput").ap() for n in "qkvo"}
    uT = {n: nc.dram_tensor("u" + n + "T", [FRAMES, RANK, C], BF,
                            kind="ExternalInput").ap() for n in "qkvo"}
    # bias rows: 0 bo, 1 bi, 2 btq*0.125, 3 btk, 4 btv, 5 bto, 6 gamma, 7 beta
    biases = nc.dram_tensor("biases", [8, C], F32, kind="ExternalInput").ap()
    outT = nc.dram_tensor("outT", [FRAMES, C, DQ], F32,
                          kind="ExternalOutput").ap()
    dbg = {}
    if DEBUG:
        for nm, shp in (("kT", [C, D]), ("v", [SC_N * P, HEADS * SLOT]),
                        ("qT", [C, DQ]), ("oT", [C, DQ]),
                        ("o", [C, FRAMES * DQ]), ("xn", [C, FRAMES * DQ]),
                        ("qtT", [C, FRAMES * DQ]),
                        ("ktT", [C, FRAMES * DQ]), ("vtT", [C, FRAMES * DQ]),
                        ("xo", [C, FRAMES * DQ])):
            dbg[nm] = nc.dram_tensor("dbg_" + nm, shp,
                                     mybir.dt.bfloat16,
                                     kind="ExternalOutput").ap()

    NCH_V = ((0, 512), (512, 512), (1024, 256))  # c_out chunks for V row-major

    with tile.TileContext(nc) as tc, ExitStack() as top:
        const = top.enter_context(tc.tile_pool(name="const", bufs=1))
        p_w = top.enter_context(tc.tile_pool(name="w", bufs=int(os.environ.get("K_W", "12"))))
        p_u = top.enter_context(tc.tile_pool(name="u", bufs=3))
        p_d = top.enter_context(tc.tile_pool(name="d", bufs=10))
        p_dram = top.enter_context(tc.tile_pool(name="dram", bufs=1,
                                                space="DRAM"))
        psA = top.enter_context(tc.tile_pool(name="psA", bufs=2,
                                             space="PSUM"))
        psS = top.enter_context(tc.tile_pool(name="psS", bufs=2,
                                             space="PSUM"))
        psC = top.enter_context(tc.tile_pool(name="psC", bufs=2,
                                             space="PSUM"))

        bias_sb = const.tile([P, 80], F32)  # col i*10+kc
        nc.sync.dma_start(bias_sb[:], biases.rearrange("i (k p) -> p (i k)",
                                                       p=P))

        def bias_ap(i, kc):
            return bias_sb[:, i * KC + kc:i * KC + kc + 1]

        ones_col2 = const.tile([P, P], BF)       # lhsT rows for k=1 broadcasts
        nc.vector.memset(ones_col2[:], 1.0)
        ones_part = const.tile([P, 1], F32)      # lhsT for partition sums
        nc.vector.memset(ones_part[:], 1.0)
        ones_part_bf = const.tile([P, 1], BF)    # bf16 variant (logit sums)
        nc.vector.memset(ones_part_bf[:], 1.0)
        ones_neg_bf = const.tile([P, 1], BF)     # -1 (fused l1-l0 accumulate)
        nc.vector.memset(ones_neg_bf[:], -1.0)
        eps_sb = const.tile([P, 1], F32)
        nc.vector.memset(eps_sb[:], EPS)

        oT_dram = p_dram.tile([C, FRAMES * DQ], BF)

        def load_w(name):
            tiles = []
            for kc in range(KC):
                t = p_w.tile([P, C], BF, tag="w")
                nc.sync.dma_start(t[:], wT[name][kc * P:(kc + 1) * P, :])
                tiles.append(t)
            return tiles

        def load_du(name, f):
            dts, uts = [], []
            for kc in range(KC):
                t = p_d.tile([P, RANK], BF, tag="d")
                nc.sync.dma_start(t[:], dT[name][f, kc * P:(kc + 1) * P, :])
                dts.append(t)
            for rc in range(RC):
                t = p_u.tile([P, C], BF, tag="u")
                nc.sync.dma_start(t[:], uT[name][f, rc * P:(rc + 1) * P, :])
                uts.append(t)
            return dts, uts

        # ============================ SPATIAL ============================
        with ExitStack() as sp:
            p_hT = sp.enter_context(tc.tile_pool(name="hT", bufs=KC))
            p_kT = sp.enter_context(tc.tile_pool(name="kT", bufs=KC))
            p_v = sp.enter_context(tc.tile_pool(name="v", bufs=SC_N))
            p_qa = sp.enter_context(tc.tile_pool(name="qa", bufs=int(os.environ.get("K_QA", "12"))))
            p_t = sp.enter_context(tc.tile_pool(name="t", bufs=int(os.environ.get("K_T", "2"))))
            p_dq = sp.enter_context(tc.tile_pool(name="dq", bufs=int(os.environ.get("K_DQ", "11"))))
            p_sm = sp.enter_context(tc.tile_pool(name="sm", bufs=2))
            p_e = sp.enter_context(tc.tile_pool(name="e", bufs=3))

            for f in range(FRAMES):
                dk, uk = load_du("k", f)
                hts = []
                for kc in range(KC):
                    t = p_hT.tile([P, D], BF, tag="ht")
                    half = D // 2
                    for hh in range(2):
                        nc.sync.dma_start(
                            t[:, hh * half:(hh + 1) * half],
                            hT[f, kc * P:(kc + 1) * P,
                               hh * half:(hh + 1) * half])
                    hts.append(t)

                # ---- lora intermediate t = D @ x^T, feature-major [RANK, D]
                def lora_t(dts, rhs_tiles, n_all, pool, tag):
                    outs = []
                    for rc in range(RC):
                        t = pool.tile([P, n_all], BF, tag=tag)
                        nchunks = n_all // DQ
                        for ncn in range(nchunks):
                            ps = psA.tile([P, DQ], F32, tag="psA")
                            for kc in range(KC):
                                nc.tensor.matmul(
                                    ps[:],
                                    dts[kc][:, rc * P:(rc + 1) * P],
                                    rhs_tiles[kc][:, ncn * DQ:(ncn + 1) * DQ],
                                    start=(kc == 0), stop=(kc == KC - 1))
                            nc.scalar.copy(t[:, ncn * DQ:(ncn + 1) * DQ],
                                           ps[:])
                        outs.append(t)
                    return outs

                # ---- K^T feature-major [C, D] (+ lora)
                tkt = lora_t(dk, hts, D, p_t, "lt")
                wk = load_w("wk")
                kts = []
                for mc in range(KC):
                    t = p_kT.tile([P, D], BF, tag="kt")
                    for ncn in range(D // DQ):
                        ps = psA.tile([P, DQ], F32, tag="psA")
                        for kc in range(KC):
                            nc.tensor.matmul(
                                ps[:], wk[kc][:, mc * P:(mc + 1) * P],
                                hts[kc][:, ncn * DQ:(ncn + 1) * DQ],
                                start=(kc == 0), stop=False)
                        for rc in range(RC):
                            nc.tensor.matmul(
                                ps[:], uk[rc][:, mc * P:(mc + 1) * P],
                                tkt[rc][:, ncn * DQ:(ncn + 1) * DQ],
                                start=False, stop=(rc == RC - 1))
                        nc.vector.tensor_copy(t[:, ncn * DQ:(ncn + 1) * DQ],
                                              ps[:])
                    kts.append(t)

                if DEBUG and f == 0:
                    for mc in range(KC):
                        nc.sync.dma_start(
                            dbg["kT"][mc * P:(mc + 1) * P, :], kts[mc][:])

                # ---- V row-major with 65-wide head slots [D, HEADS*SLOT]
                dv, uv = load_du("v", f)
                tvt = lora_t(dv, hts, D, p_t, "lt")
                wv = load_w("wv")
                vts = []
                for mc in range(SC_N):
                    t = p_v.tile([P, HEADS * SLOT], BF, tag="v")
                    v3 = t[:, :].rearrange("p (h x) -> p h x", x=SLOT)
                    nc.vector.memset(v3[:, :, DH:SLOT], 1.0)
                    for (n0, nw) in NCH_V:
                        ps = psA.tile([P, DQ], F32, tag="psA")
                        for kc in range(KC):
                            nc.tensor.matmul(
                                ps[:, 0:nw],
                                hts[kc][:, mc * P:(mc + 1) * P],
                                wv[kc][:, n0:n0 + nw],
                                start=(kc == 0), stop=False)
                        for rc in range(RC):
                            nc.tensor.matmul(
                                ps[:, 0:nw],
                                tvt[rc][:, mc * P:(mc + 1) * P],
                                uv[rc][:, n0:n0 + nw],
                                start=False, stop=(rc == RC - 1))
                        h0 = n0 // DH
                        nc.scalar.copy(
                            v3[:, h0:h0 + nw // DH, 0:DH],
                            ps[:, 0:nw].rearrange("p (h x) -> p h x", x=DH))
                    vts.append(t)

                if DEBUG and f == 0:
                    for mc in range(SC_N):
                        nc.sync.dma_start(
                            dbg["v"][mc * P:(mc + 1) * P, :], vts[mc][:])

                # ---- Q^T feature-major [C, DQ], prescaled by dh^-0.5
                hqs = []
                for kc in range(KC):
                    t = p_dq.tile([P, DQ], BF, tag="dq")
                    nc.sync.dma_start(t[:], hTq[f, kc * P:(kc + 1) * P, :])
                    hqs.append(t)
                dq_, uq = load_du("q", f)
                tqt = lora_t(dq_, hqs, DQ, p_t, "lt")
                wq = load_w("wq")
                qts = []
                for mc in range(KC):
                    t = p_qa.tile([P, DQ], BF, tag="qa", name="qt")
                    ps = psA.tile([P, DQ], F32, tag="psA")
                    for kc in range(KC):
                        nc.tensor.matmul(ps[:], wq[kc][:, mc * P:(mc + 1) * P],
                                         hqs[kc], start=(kc == 0), stop=False)
                    for rc in range(RC):
                        nc.tensor.matmul(ps[:], uq[rc][:, mc * P:(mc + 1) * P],
                                         tqt[rc][:], start=False,
                                         stop=(rc == RC - 1))
                    nc.scalar.mul(t[:], ps[:], float(DH) ** -0.5)
                    qts.append(t)

                if DEBUG and f == 0:
                    for mc in range(KC):
                        nc.sync.dma_start(
                            dbg["qT"][mc * P:(mc + 1) * P, :], qts[mc][:])

                # ---- attention, head by head -> O^T feature-major [C, DQ]
                ots = [None] * KC
                for h in range(HEADS):
                    kct, off = h // 2, DH * (h % 2)
                    if ots[kct] is None:
                        ots[kct] = p_qa.tile([P, DQ], BF, tag="qa", name="ot")
                    po = psC.tile([P, DQ], F32, tag="psC")
                    for sc in range(0, SC_N, 2):
                        # two key-chunks share one 2-bank psum tile so a
                        # single exp op covers both (halves ACT op count)
                        ps = psS.tile([P, 2 * DQ], F32, tag="psS")
                        for j in range(2):
                            nc.tensor.matmul(
                                ps[:, j * DQ:(j + 1) * DQ],
                                kts[kct][off:off + DH,
                                         (sc + j) * P:(sc + j + 1) * P],
                                qts[kct][off:off + DH, :],
                                start=True, stop=True)
                        e = p_e.tile([P, 2 * DQ], BF, tag="e")
                        nc.scalar.activation(e[:], ps[:], AF.Exp)
                        for j in range(2):
                            nc.tensor.matmul(
                                po[0:SLOT, :],
                                vts[sc + j][:, h * SLOT:(h + 1) * SLOT],
                                e[:, j * DQ:(j + 1) * DQ],
                                start=(sc + j == 0),
                                stop=(sc + j == SC_N - 1))
                    rec = p_sm.tile([P, DQ], F32, tag="rec")
                    nc.vector.reciprocal(rec[DH:SLOT, :], po[DH:SLOT, :])
                    recb = p_sm.tile([P, DQ], BF, tag="recb")
                    nc.vector.tensor_copy(recb[DH:SLOT, :], rec[DH:SLOT, :])
                    pb = psA.tile([P, DQ], F32, tag="psA", name="pb")
                    nc.tensor.matmul(pb[0:DH, :], ones_col2[DH:DH + 1, 0:DH],
                                     recb[DH:SLOT, :], start=True, stop=True)
                    pbs = p_dq.tile([P, DQ], BF, tag="dq", name="pbs")
                    nc.vector.tensor_copy(pbs[0:DH, :], pb[0:DH, :])
                    if off == 0:
                        nc.vector.tensor_mul(ots[kct][0:DH, :],
                                             po[0:DH, :], pbs[0:DH, :])
                    else:
                        onrm = p_dq.tile([P, DQ], BF, tag="dq", name="onrm")
                        nc.vector.tensor_mul(onrm[0:DH, :],
                                             po[0:DH, :], pbs[0:DH, :])
                        nc.sync.dma_start(ots[kct][DH:P, :], onrm[0:DH, :])

                if DEBUG and f == 0:
                    for mc in range(KC):
                        nc.sync.dma_start(
                            dbg["oT"][mc * P:(mc + 1) * P, :], ots[mc][:])

                # ---- O-projection (+ lora + bo) -> oT_dram
                do, uo = load_du("o", f)
                tot = lora_t(do, ots, DQ, p_t, "lt")
                wo = load_w("wo")
                for mc in range(KC):
                    ps = psA.tile([P, DQ], F32, tag="psA")
                    for kc in range(KC):
                        nc.tensor.matmul(ps[:], wo[kc][:, mc * P:(mc + 1) * P],
                                         ots[kc], start=(kc == 0), stop=False)
                    for rc in range(RC):
                        nc.tensor.matmul(ps[:], uo[rc][:, mc * P:(mc + 1) * P],
                                         tot[rc][:], start=False,
                                         stop=(rc == RC - 1))
                    ot_e = p_dq.tile([P, DQ], BF, tag="dq")
                    nc.vector.tensor_scalar_add(ot_e[:], ps[:], bias_ap(0, mc))
                    nc.sync.dma_start(
                        oT_dram[mc * P:(mc + 1) * P, f * DQ:(f + 1) * DQ],
                        ot_e[:])
                    if DEBUG:
                        nc.sync.dma_start(
                            dbg["o"][mc * P:(mc + 1) * P,
                                     f * DQ:(f + 1) * DQ], ot_e[:])

        # ============================ TEMPORAL ===========================
        D2 = FRAMES * DQ  # 1024 temporal rows (f-major columns)
        with ExitStack() as tp:
            p_act = tp.enter_context(tc.tile_pool(name="act", bufs=40))
            p_o2 = tp.enter_context(tc.tile_pool(name="o2", bufs=KC))
            p_mhf = tp.enter_context(tc.tile_pool(name="mhf", bufs=4))
            p_h2 = tp.enter_context(tc.tile_pool(name="h2", bufs=2))
            p_vd = tp.enter_context(tc.tile_pool(name="vd", bufs=KC))
            p_pr = tp.enter_context(tc.tile_pool(name="pr", bufs=4))
            p_fi = tp.enter_context(tc.tile_pool(name="fi", bufs=4))
            p_sm2 = tp.enter_context(tc.tile_pool(name="sm2", bufs=4))
            p_pp = tp.enter_context(tc.tile_pool(name="pp", bufs=4))

            o2s = []
            for kc in range(KC):
                t = p_o2.tile([P, D2], BF, tag="o2")
                nc.sync.dma_start(t[:], oT_dram[kc * P:(kc + 1) * P, :])
                o2s.append(t)

            def load_h2(kc):
                t = p_h2.tile([P, D2], BF, tag="h2")
                nc.sync.dma_start(
                    t[:, :].rearrange("p (f n) -> p f n", f=FRAMES),
                    hTq[:, kc * P:(kc + 1) * P, :].rearrange(
                        "f p n -> p f n"))
                return t

            # ---- LN stats over c (partition-dim sums via ones matmuls)
            stat_pool = [psS, psS, psC, psC]
            pmu = [stat_pool[n].tile([P, DQ], F32, tag=stat_pool[n].name,
                                            name="pmu") for n in range(2)]
            psq = [stat_pool[2 + n].tile([P, DQ], F32, name="psq",
                                         tag=stat_pool[2 + n].name)
                   for n in range(2)]
            for kc in range(KC):
                h2 = load_h2(kc)
                mhf = p_mhf.tile([P, D2], BF, tag="mhf")
                nc.vector.tensor_add(mhf[:], o2s[kc][:], h2[:])
                sq = p_mhf.tile([P, D2], BF, tag="mhf")
                nc.scalar.square(sq[:], mhf[:])
                for n in range(2):
                    nc.tensor.matmul(pmu[n][0:1, :], ones_part_bf[:],
                                     mhf[:, n * DQ:(n + 1) * DQ],
                                     start=(kc == 0), stop=(kc == KC - 1))
                    nc.tensor.matmul(psq[n][0:1, :], ones_part_bf[:],
                                     sq[:, n * DQ:(n + 1) * DQ],
                                     start=(kc == 0), stop=(kc == KC - 1))
            mu = p_sm2.tile([1, D2], F32, tag="st")
            rsd = p_sm2.tile([1, D2], F32, tag="st")
            var = p_sm2.tile([1, D2], F32, tag="st")
            for n in range(2):
                sl = slice(n * DQ, (n + 1) * DQ)
                nc.scalar.mul(mu[:, sl], pmu[n][0:1, :], 1.0 / C)
                nc.scalar.mul(var[:, sl], psq[n][0:1, :], 1.0 / C)
            musq = p_sm2.tile([1, D2], F32, tag="st")
            nc.vector.tensor_mul(musq[:], mu[:], mu[:])
            nc.vector.tensor_sub(var[:], var[:], musq[:])
            sd = p_sm2.tile([1, D2], F32, tag="st")
            nc.scalar.activation(sd[:], var[:], AF.Sqrt, bias=eps_sb[0:1, :])
            nc.vector.reciprocal(rsd[:], sd[:])
            # broadcast mu and 1/sd across partitions (bf16 matmuls)
            mub = p_sm2.tile([1, D2], BF, tag="stb")
            nc.vector.tensor_copy(mub[:], mu[:])
            rsdb = p_sm2.tile([1, D2], BF, tag="stb")
            nc.vector.tensor_copy(rsdb[:], rsd[:])
            bmu, brs = [], []
            for n in range(2):
                sl = slice(n * DQ, (n + 1) * DQ)
                t = psS.tile([P, DQ], F32, tag="psS", name="bmu")
                nc.tensor.matmul(t[:], ones_col2[0:1, :], mub[:, sl],
                                 start=True, stop=True)
                bmu.append(t)
                t = psC.tile([P, DQ], F32, tag="psC")
                nc.tensor.matmul(t[:], ones_col2[0:1, :], rsdb[:, sl],
                                 start=True, stop=True)
                brs.append(t)

            # ---- xn = (mhf - mu) * rsd * gamma + beta  (bf16)
            xns = []
            for kc in range(KC):
                h2 = load_h2(kc)
                mhf = p_mhf.tile([P, D2], BF, tag="mhf")
                nc.vector.tensor_add(mhf[:], o2s[kc][:], h2[:])
                xn = p_act.tile([P, D2], BF, tag="act")
                for n in range(2):
                    sl = slice(n * DQ, (n + 1) * DQ)
                    t1 = p_fi.tile([P, DQ], F32, tag="fi")
                    nc.vector.tensor_sub(t1[:], mhf[:, sl], bmu[n][:])
                    nc.vector.tensor_mul(t1[:], t1[:], brs[n][:])
                    nc.scalar.activation(xn[:, sl], t1[:], AF.Identity,
                                         bias=bias_ap(7, kc),
                                         scale=bias_ap(6, kc))
                xns.append(xn)
                if DEBUG:
                    nc.sync.dma_start(dbg["xn"][kc * P:(kc + 1) * P, :],
                                      xn[:])

            def t_proj(name, src, bias_i, act_scale=None):
                w = load_w(name)
                outs = []
                for mc in range(KC):
                    t = p_act.tile([P, D2], BF, tag="act")
                    for n in range(2):
                        sl = slice(n * DQ, (n + 1) * DQ)
                        ps = psA.tile([P, DQ], F32, tag="psA")
                        for kc in range(KC):
                            nc.tensor.matmul(
                                ps[:], w[kc][:, mc * P:(mc + 1) * P],
                                src[kc][:, sl],
                                start=(kc == 0), stop=(kc == KC - 1))
                        if act_scale is not None:
                            nc.scalar.activation(t[:, sl], ps[:], AF.Identity,
                                                 bias=bias_ap(bias_i, mc),
                                                 scale=act_scale)
                        else:
                            nc.vector.tensor_scalar_add(t[:, sl], ps[:],
                                                        bias_ap(bias_i, mc))
                    outs.append(t)
                return outs

            # qt/kt/vt directly from xn: host fused W' = Wt{q,k,v} @ Wi and
            # b' = Wt{q,k,v} @ bi + bt{q,k,v} (xi is consumed only by these).
            qts = t_proj("wtq", xns, 2, act_scale=float(DH) ** -0.5)
            kts = t_proj("wtk", xns, 3)
            vts = t_proj("wtv", xns, 4)
            if DEBUG:
                for kc in range(KC):
                    nc.sync.dma_start(dbg["qtT"][kc * P:(kc + 1) * P, :],
                                      qts[kc][:])
                    nc.sync.dma_start(dbg["ktT"][kc * P:(kc + 1) * P, :],
                                      kts[kc][:])
                    nc.sync.dma_start(dbg["vtT"][kc * P:(kc + 1) * P, :],
                                      vts[kc][:])

            # ---- temporal attention over f=2 (sigmoid trick)
            vds = []
            for kc in range(KC):
                t = p_vd.tile([P, DQ], BF, tag="vd")
                nc.vector.tensor_sub(t[:], vts[kc][:, DQ:D2],
                                     vts[kc][:, 0:DQ])
                vds.append(t)
            xos = []
            for kc in range(KC):
                xos.append(p_act.tile([P, D2], BF, tag="act", name="xo"))
            for kc in range(KC):
                prods = {}
                for (i, j) in ((0, 0), (0, 1), (1, 0), (1, 1)):
                    pr = p_pr.tile([P, DQ], BF, tag="pr")
                    nc.vector.tensor_mul(pr[:],
                                         qts[kc][:, i * DQ:(i + 1) * DQ],
                                         kts[kc][:, j * DQ:(j + 1) * DQ])
                    prods[(i, j)] = pr
                for h2i in range(2):
                    off = DH * h2i
                    osl = slice(off, off + DH)
                    pl = psC.tile([P, DQ], F32, tag="psC", name="pl")
                    for i in range(2):
                        row = i * 64
                        nc.tensor.matmul(pl[row:row + 1, :],
                                         ones_neg_bf[osl, :],
                                         prods[(i, 0)][osl, :],
                                         start=True, stop=False)
                        nc.tensor.matmul(pl[row:row + 1, :],
                                         ones_part_bf[osl, :],
                                         prods[(i, 1)][osl, :],
                                         start=False, stop=True)
                    for i in range(2):
                        rp = i * 64
                        pp = p_pp.tile([P, DQ], BF, tag="pp")
                        nc.scalar.activation(pp[rp:rp + 1, :],
                                             pl[rp:rp + 1, :], AF.Sigmoid)
                        pb = psA.tile([P, DQ], F32, tag="psA", name="pb2")
                        nc.tensor.matmul(pb[off:off + DH, :],
                                         ones_col2[rp:rp + 1, 0:DH],
                                         pp[rp:rp + 1, :],
                                         start=True, stop=True)
                        tm = p_fi.tile([P, DQ], F32, tag="fi")
                        nc.vector.tensor_mul(tm[osl, :], pb[off:off + DH, :],
                                             vds[kc][osl, :])
                        nc.vector.tensor_add(
                            xos[kc][osl, i * DQ:(i + 1) * DQ],
                            tm[osl, :], vts[kc][osl, 0:DQ])

            if DEBUG:
                for kc in range(KC):
                    nc.sync.dma_start(dbg["xo"][kc * P:(kc + 1) * P, :],
                                      xos[kc][:])

            # ---- to_out projection + bto + o (= mhf - h) -> final output
            wto = load_w("wto")
            for mc in range(KC):
                for n in range(2):
                    sl = slice(n * DQ, (n + 1) * DQ)
                    ps = psA.tile([P, DQ], F32, tag="psA")
                    for kc in range(KC):
                        nc.tensor.matmul(ps[:], wto[kc][:, mc * P:(mc + 1) * P],
                                         xos[kc][:, sl],
                                         start=(kc == 0), stop=(kc == KC - 1))
                    fin = p_fi.tile([P, DQ], F32, tag="fi")
                    nc.vector.scalar_tensor_tensor(
                        fin[:], ps[:], bias_ap(5, mc), o2s[mc][:, sl],
                        op0=mybir.AluOpType.add, op1=mybir.AluOpType.add)
                    nc.sync.dma_start(outT[n, mc * P:(mc + 1) * P, :], fin[:])

    nc.compile()
    # Strip per-instruction debug info (source file/line). It is embedded in
    # the serialized BIR inside the HLO backend config, so leaving it in
    # makes the neuron compile-cache key depend on this file's path.
    for fn in nc.m.functions:
        for bb in fn.blocks:
            for ins in bb.instructions:
                if ins.debug is not None:
                    ins.debug = None
        for alloc in fn.allocations:
            for ml in getattr(alloc, "memorylocations", None) or []:
                if getattr(ml, "ant_debug", None) is not None:
                    ml.ant_debug = None
    _CACHE["nc"] = nc
    return nc


def _prep_common(Wq, Wk, Wv, Wo, bo, Dq, Uq, Dk, Uk, Dv, Uv, Do, Uo,
                 gamma, beta, Wi, bi, Wtq, btq, Wtk, btk, Wtv, btv, Wto, bto):
    """Host-side prep of replicated tensors: transposes + bf16 casts."""
    scale = np.float32(DH ** -0.5)
    com = {
        "wqT": np.ascontiguousarray(np.asarray(Wq, np.float32).T).astype(BF16),
        "wkT": np.ascontiguousarray(np.asarray(Wk, np.float32).T).astype(BF16),
        "wvT": np.ascontiguousarray(np.asarray(Wv, np.float32).T).astype(BF16),
        "woT": np.ascontiguousarray(np.asarray(Wo, np.float32).T).astype(BF16),
        "wtoT": np.ascontiguousarray(np.asarray(Wto, np.float32).T).astype(BF16),
    }
    Wi = np.asarray(Wi, np.float32)
    bi = np.asarray(bi, np.float32)
    for nm, Wt, bt in (("wtq", Wtq, btq), ("wtk", Wtk, btk),
                       ("wtv", Wtv, btv)):
        Wt = np.asarray(Wt, np.float32)
        com[nm + "T"] = np.ascontiguousarray((Wt @ Wi).T).astype(BF16)
        com["b_" + nm] = Wt @ bi + np.asarray(bt, np.float32)
    com["biases"] = np.ascontiguousarray(np.stack([
        np.asarray(bo, np.float32),
        np.zeros(C, np.float32),
        com.pop("b_wtq") * scale,
        com.pop("b_wtk"),
        com.pop("b_wtv"),
        np.asarray(bto, np.float32),
        np.asarray(gamma, np.float32),
        np.asarray(beta, np.float32)]))
    for n, Dm, Um in (("q", Dq, Uq), ("k", Dk, Uk), ("v", Dv, Uv),
                      ("o", Do, Uo)):
        Dm = np.asarray(Dm, np.float32)
        Um = np.asarray(Um, np.float32)
        com["d%sT" % n] = np.ascontiguousarray(
            Dm.transpose(0, 2, 1)).astype(BF16)   # [F, C, RANK]
        com["u%sT" % n] = np.ascontiguousarray(
            Um.transpose(0, 2, 1)).astype(BF16)   # [F, RANK, C]
    return com


def _prep_percore(h):
    """Per-core feature-major h slices, stacked along axis 0 for shard_map."""
    h = np.asarray(h, np.float32)
    h4 = h.reshape(B, FRAMES, D, C)
    hTs, hTqs = [], []
    for core in range(NCORES):
        b, dq = core // 4, core % 4
        hT_b = np.ascontiguousarray(h4[b].transpose(0, 2, 1)).astype(BF16)
        hTs.append(hT_b)
        hTqs.append(np.ascontiguousarray(hT_b[:, :, dq * DQ:(dq + 1) * DQ]))
    return np.concatenate(hTs, 0), np.concatenate(hTqs, 0)


def _get_runner():
    """Cached shard_map runner: replicated weights, per-core h shards."""
    if "runner" in _CACHE:
        return _CACHE["runner"]
    import jax
    try:
        jax.config.update("jax_compilation_cache_dir", "/root/.jax_cache")
        jax.config.update("jax_persistent_cache_min_compile_time_secs", 0.5)
    except Exception:
        pass
    import jax.numpy as jnp
    from jax.sharding import Mesh, PartitionSpec
    from jax.experimental.shard_map import shard_map
    from concourse.bass2jax import (_bass_exec_p, install_neuronx_cc_hook,
                                    partition_id_tensor)
    from concourse import mybir

    nc = _build()
    install_neuronx_cc_hook()
    pname = nc.partition_id_tensor.name if nc.partition_id_tensor else None
    in_names, out_names, out_avals = [], [], []
    for alloc in nc.m.functions[0].allocations:
        if not isinstance(alloc, mybir.MemoryLocationSet):
            continue
        name = alloc.memorylocations[0].name
        if alloc.kind == "ExternalInput":
            if name != pname:
                in_names.append(name)
        elif alloc.kind == "ExternalOutput":
            out_names.append(name)
            out_avals.append(jax.core.ShapedArray(
                tuple(alloc.tensor_shape), mybir.dt.np(alloc.dtype)))
    n_params = len(in_names)
    all_names = in_names + out_names + ([pname] if pname else [])
    PERCORE = {"hT", "hTq"}

    def _body(*args):
        operands = list(args)
        if pname is not None:
            operands.append(partition_id_tensor())
        return tuple(_bass_exec_p.bind(
            *operands, out_avals=tuple(out_avals), in_names=tuple(all_names),
            out_names=tuple(out_names), lowering_input_output_aliases=(),
            sim_require_finite=True, sim_require_nnan=True, nc=nc))

    mesh = Mesh(np.asarray(jax.devices()[:NCORES]), ("core",))
    in_specs = tuple(PartitionSpec("core") if nm in PERCORE
                     else PartitionSpec() for nm in in_names) \
        + (PartitionSpec("core"),) * len(out_names)
    out_specs = (PartitionSpec("core"),) * len(out_names)
    donate = tuple(range(n_params, n_params + len(out_avals)))
    sharded = jax.jit(
        shard_map(_body, mesh=mesh, in_specs=in_specs, out_specs=out_specs,
                  check_rep=False),
        donate_argnums=donate, keep_unused=True)
    runner = dict(sharded=sharded, in_names=in_names, out_names=out_names,
                  out_avals=out_avals, jax=jax)
    _CACHE["runner"] = runner
    return runner


def kernel(**inputs):
    r = _get_runner()
    com = _prep_common(**{k: v for k, v in inputs.items() if k != "h"})
    hT_cat, hTq_cat = _prep_percore(inputs["h"])
    percore = {"hT": hT_cat, "hTq": hTq_cat}
    args = [percore[nm] if nm in percore else com[nm]
            for nm in r["in_names"]]
    zeros = [np.zeros((NCORES * av.shape[0], *av.shape[1:]), av.dtype)
             for av in r["out_avals"]]
    outs = r["sharded"](*args, *zeros)
    out_cat = np.asarray(outs[r["out_names"].index("outT")])
    out_cat = out_cat.reshape(NCORES, FRAMES, C, DQ)
    out = np.empty([B * FRAMES, D, C], np.float32)
    for core in range(NCORES):
        b, dq = core // 4, core % 4
        for f in range(FRAMES):
            out[b * FRAMES + f, dq * DQ:(dq + 1) * DQ, :] = \
                out_cat[core, f].T
    return out


def _prep_inputs(**inputs):
    """Back-compat helper for sim tests: per-core in_maps."""
    com = _prep_common(**{k: v for k, v in inputs.items() if k != "h"})
    hT_cat, hTq_cat = _prep_percore(inputs["h"])
    hT = hT_cat.reshape(NCORES, FRAMES, C, D)
    hTq = hTq_cat.reshape(NCORES, FRAMES, C, DQ)
    return [dict(com, hT=hT[c], hTq=hTq[c]) for c in range(NCORES)]
